# revision 24
# baseline (speedup 1.0000x reference)
"""Trainium2 Bass kernel for nn_MemoryEfficientBSpline (linear B-spline / KAN layer).

Math: out[b,o,p] = sum_i sum_g Wt[b,i,p,g] * coef[b,o,i,g] where Wt is the
two-hot linear-interpolation weight of x[b,i,p] over a 6-knot grid on [-1,1].

Reformulation (hinge basis): the piecewise-linear spline becomes 5 dense
[64x64] matmuls over clip planes of x:

  out[b,o,p] = alpha[b,o] + sum_{k=0..4} sum_i Wk[b,o,i]*clip(x[b,i,p], t_k, 1)

with knots t = [-1, -0.6, -0.2, 0.2, 0.6]; the hinge shift relu(xc-t) =
clip(x,t,1) - t is folded into alpha host-side. All 5 planes are a single
DVE tensor_scalar (min,max) op each — no ACT dependency chain.

Sharding: data-parallel over batch B=8, one batch per NeuronCore. Per core the
64x36864 pixel plane is folded to 128 partitions (two pixel-halves stacked), and
the 64x64 weights are block-diagonal-duplicated to 128x128 so one full-array
matmul handles both halves.

Dtype: fp16 end-to-end (x in, planes, weights, y out; f32 psum accumulate).
Halves HBM traffic vs f32 (the 8-core steady state is chip-HBM-bound) and
doubles DVE throughput (4x mode for 16-bit SBUF operands). Rel err ~2e-3.

Schedule: PE warm-up dummy matmuls (on a memset scratch tile) cover the
~2.5us first-DMA latency and the DVFS p-state ramp; slab sizes ramp
128/384/1024 then 8x2048 with a 512 tail for a short drain.
"""
import numpy as np
from contextlib import ExitStack

import concourse.bass as bass
import concourse.tile as tile
from concourse import bacc, mybir
from concourse.bass_utils import run_bass_kernel_spmd

# Problem shapes (hardcoded per contract)
B, IN_DIM, H, W = 8, 64, 192, 192
OUT_DIM = 64
G = 6
P_TOT = H * W          # 36864 pixels
HALF = P_TOT // 2      # 18432 (folded columns)
NPART = 128
CHUNK = 512            # matmul moving-operand size (= one PSUM bank of fp32)
MAXSLAB = 2048
# Head ramps up so the first planes/matmuls start ASAP after the ~2.5us
# first-DMA latency; tail ramps down so the last evict+store drain is short.
SLAB_SIZES = [256, 512, 1024] + [2048] * 7 + [1024, 768, 256, 256]  # sum 18432
NK = 5                 # clip planes: t = -1, -0.6, -0.2, 0.2, 0.6
KNOTS = (-1.0, -0.6, -0.2, 0.2, 0.6)
N_DUMMY = 6            # PE warm-up matmuls (ramp + fill masking; any PE idle
                       # gap before the first real matmul resets the p-state
                       # ramp, so slightly over-covering the fill is cheaper)

_f16 = mybir.dt.float16
_f32 = mybir.dt.float32
_Alu = mybir.AluOpType
_Act = mybir.ActivationFunctionType

_cached = None  # compiled Bass module, built once per process


def _build_module(n_reps=1):
    """n_reps>1 wraps the slab loop in-line — used only for slope-based HW
    timing (dispatch noise >> exec time in this env)."""
    nc = bacc.Bacc("TRN2", target_bir_lowering=False, debug=False,
                   enable_asserts=False, num_devices=8)

    x_t = nc.dram_tensor("x", (NPART, HALF), _f16, kind="ExternalInput")
    w_t = nc.dram_tensor("wts", (NPART, NK * NPART), _f16, kind="ExternalInput")
    b_t = nc.dram_tensor("bias", (NPART, 1), _f32, kind="ExternalInput")
    y_t = nc.dram_tensor("y", (NPART, HALF), _f16, kind="ExternalOutput")

    with tile.TileContext(nc) as tc, ExitStack() as ctx:
        cpool = ctx.enter_context(tc.tile_pool(name="const", bufs=1))
        xpool = ctx.enter_context(tc.tile_pool(name="xin", bufs=4))
        ppool = ctx.enter_context(tc.tile_pool(name="planes", bufs=3))
        opool = ctx.enter_context(tc.tile_pool(name="oslab", bufs=4))
        psum = ctx.enter_context(tc.tile_pool(name="acc", bufs=1, space="PSUM"))

        # Warm-up scratch: DVE memset is ready ~0.4us in, long before any DMA
        # lands, so dummy matmuls on it keep the PE busy (and ramping to full
        # clock) while the first x slab + weights are still in flight.
        warm = cpool.tile([NPART, CHUNK], _f16)
        nc.vector.memset(warm[:], 0.0)

        # HWDGE configs serialize globally, so DMA issue order is critical:
        # weights first as ONE transfer (its sem gates every matmul; a split
        # serializes configs and delays k>=1 matmuls ~1.5us), slab-0 x second,
        # bias via the Pool queue's software DGE, which skips HWDGE entirely
        # (it's only needed by the first evict, ~8us in).
        wts = cpool.tile([NPART, NK * NPART], _f16)
        nc.sync.dma_start(wts[:], w_t[:])
        bias = cpool.tile([NPART, 1], _f32)
        nc.gpsimd.dma_start(bias[:], b_t[:])

        acc0 = psum.tile([NPART, MAXSLAB], _f32, tag="acc0", name="acc0")
        acc1 = psum.tile([NPART, MAXSLAB], _f32, tag="acc1", name="acc1")
        accs = [acc0, acc1]

        for d in range(N_DUMMY):
            nc.tensor.matmul(acc0[:, :CHUNK], warm[:, :NPART], warm[:],
                             start=True, stop=True)

        n_slabs = len(SLAB_SIZES)
        col_starts = np.cumsum([0] + SLAB_SIZES).tolist()
        col_starts.append(col_starts[-1])  # sentinel for load_slab(s+1) slice

        PREFETCH = 2    # slabs of load lead over the store stream on SP

        def load_slab(s):
            xt = xpool.tile([NPART, MAXSLAB], _f16, tag="x", name="xt")
            nc.sync.dma_start(xt[:, :SLAB_SIZES[s]],
                              x_t[:, col_starts[s]:col_starts[s + 1]])
            return xt

        def body():
            pending = [load_slab(s) for s in range(PREFETCH)]
            for s in range(n_slabs):
                col0, sz = col_starts[s], SLAB_SIZES[s]
                if s + PREFETCH < n_slabs:
                    pending.append(load_slab(s + PREFETCH))
                xt = pending.pop(0)

                # 5 clip planes, all independent single DVE ops on raw x
                planes = []
                for k, t in enumerate(KNOTS):
                    pk = ppool.tile([NPART, MAXSLAB], _f16, tag=f"p{k}",
                                    name=f"p{k}")
                    nc.vector.tensor_scalar(pk[:, :sz], xt[:, :sz], 1.0, t,
                                            _Alu.min, _Alu.max)
                    planes.append(pk)

                # Matmuls: 5 planes x 512-chunks accumulating in psum
                acc = accs[s % 2]
                for k in range(NK):
                    wk = wts[:, k * NPART:(k + 1) * NPART]
                    for c0 in range(0, sz, CHUNK):
                        w = min(CHUNK, sz - c0)
                        nc.tensor.matmul(acc[:, c0:c0 + w], wk,
                                         planes[k][:, c0:c0 + w],
                                         start=(k == 0), stop=(k == NK - 1))

                # Evict + bias in one ACT pass, then DMA out on SP. (Issuing
                # the store from ACT looks tempting but its DMA config blocks
                # the next evict on the ACT sequencer — measured worse.) The
                # LAST slab evicts on DVE instead: DVE is idle by then (all
                # planes done), while ACT still has the previous slab's evict
                # in flight — the two tail evicts run in parallel.
                ot = opool.tile([NPART, MAXSLAB], _f16, tag="o", name="ot")
                if s == n_slabs - 1:
                    nc.vector.tensor_scalar(ot[:, :sz], acc[:, :sz], bias[:],
                                            None, _Alu.add)
                else:
                    nc.scalar.activation(ot[:, :sz], acc[:, :sz], _Act.Identity,
                                         bias=bias[:], scale=1.0)
                nc.sync.dma_start(y_t[:, col0:col0 + sz], ot[:, :sz])

        for r in range(n_reps):
            body()

    nc.compile()
    return nc


def _get_module():
    global _cached
    if _cached is None:
        _cached = _build_module()
    return _cached


def _prep_inputs(x, coef):
    """Host-side shard + coefficient transform. Returns in_maps for 8 cores."""
    x16 = np.asarray(x, dtype=np.float16)             # [B, i, H, W]
    c = np.asarray(coef, dtype=np.float64)            # [B, o, i, 6]
    d = np.diff(c, axis=-1)                           # [B, o, i, 5]
    beta = np.concatenate([d[..., :1], np.diff(d, axis=-1)], axis=-1)
    Wk = (2.5 * beta).astype(np.float16)              # [B, o, i, 5]
    Wk64 = Wk.astype(np.float64)
    # Device plane k is clip(x, t_k, 1) = relu(xc - t_k) + t_k: fold the
    # +t_k shift into the output bias, using the fp16-rounded weights.
    alpha = (c[..., 0].sum(axis=2)
             - sum(t * Wk64[..., k].sum(axis=2) for k, t in enumerate(KNOTS))
             ).astype(np.float32)                     # [B, o]

    in_maps = []
    eye2 = np.eye(2, dtype=np.float16)
    for b in range(B):
        xb = x16[b].reshape(IN_DIM, P_TOT)
        x_f = np.concatenate([xb[:, :HALF], xb[:, HALF:]], axis=0)  # [128, HALF]
        # lhsT[k][i, o] = Wk[b, o, i, k], block-diag duplicated to 128x128,
        # packed as one contiguous [128, 5*128] tensor (single DMA)
        lhsT = np.einsum('oik->kio', Wk[b])           # [5, i, o]
        blocks = np.kron(eye2, lhsT)                  # [5, 128, 128]
        wts = np.ascontiguousarray(
            np.transpose(blocks, (1, 0, 2)).reshape(NPART, NK * NPART))
        bias = np.tile(alpha[b], 2).reshape(NPART, 1).astype(np.float32)
        in_maps.append({
            "x": np.ascontiguousarray(x_f),
            "wts": wts,
            "bias": bias,
        })
    return in_maps


def _assemble(results):
    out = np.empty((B, OUT_DIM, H, W), dtype=np.float32)
    for b in range(B):
        y_f = results[b]["y"].astype(np.float32)       # [128, HALF]
        out[b] = np.concatenate([y_f[:OUT_DIM], y_f[OUT_DIM:]], axis=1).reshape(OUT_DIM, H, W)
    return out


def run(x, coef, **spmd_kwargs):
    """Run on 8 NeuronCores; returns (output, BassKernelResults)."""
    nc = _get_module()
    in_maps = _prep_inputs(x, coef)
    res = run_bass_kernel_spmd(nc, in_maps, core_ids=list(range(8)), **spmd_kwargs)
    return _assemble(res.results), res


def kernel(x, coef):
    out, _ = run(x, coef)
    return out
